# revision 1
# baseline (speedup 1.0000x reference)
"""Trainium2 kernel for CFA-style KNN retrieval scoring.

Computes, for each row of phi [B*HW, C]:
  d_m = sqrt(|phi|^2 + |c_m|^2 - 2 phi.c_m)  over M bank entries,
  top-3 smallest distances d0<=d1<=d2,
  score = d0 / (1 + exp(d0-d1) + exp(d0-d2))      (= softmin weight * d0)

Strategy (8 NeuronCores, data-parallel over rows):
 - shard rows (B*HW = 50176) into 8 contiguous chunks of 6272 rows
 - bf16 matmul on TensorE (fp32 PSUM accumulate); the -0.5*|c_m|^2 term is
   folded into the accumulation via a K=2 ones-matmul with a bf16 hi/lo
   split of the centers row (keeps its error ~1e-3 instead of bf16 ulp ~4)
 - selection runs on psum = phi.c - 0.5|c|^2 directly (|phi|^2 is constant
   per row, monotone under sqrt -> does not affect the ranking); DVE `max`
   (top-8) per 448-wide PSUM tile, then a second `max` over 56 candidates
 - |phi|^2 computed in fp32 on ScalarE (activation Square + accum)
 - final softmin math on 3 values/row at the end, batched over all tiles
"""

import sys

import numpy as np

if "/opt/trn_rl_repo" not in sys.path:
    sys.path.insert(0, "/opt/trn_rl_repo")

B, HW, C, M = 16, 3136, 1792, 3136
NCORES = 8
ROWS = B * HW // NCORES     # 6272 rows per core
P = 128                     # partitions
NT = ROWS // P              # 49 row-tiles per core; row = p*NT + t
KC = C // P                 # 14 contraction chunks
MT = 448                    # matmul moving free size (one PSUM bank)
NMT = M // MT               # 7 m-tiles

_CACHE = {}
# selected by benchmarking; see _build_program4
KERNEL_KW = dict(groups=(3, 4))
KERNEL_GROUPS = (3, 4)   # kept for compat with older test.py paths


def _build_program(nt=NT, reps=1):
    import contextlib
    import concourse.mybir as mybir
    from concourse import bacc
    from concourse.tile import TileContext
    from concourse.masks import make_identity

    f32 = mybir.dt.float32
    bf16 = mybir.dt.bfloat16
    rows = P * nt

    nc = bacc.Bacc("TRN2", target_bir_lowering=False, debug=False)
    phi = nc.dram_tensor("phi", [rows, C], f32, kind="ExternalInput")
    cbank = nc.dram_tensor("cbank", [C, M], bf16, kind="ExternalInput")
    cc2 = nc.dram_tensor("cc2", [2, M], bf16, kind="ExternalInput")
    out = nc.dram_tensor("out", [rows, 1], f32, kind="ExternalOutput")

    phi_r = phi[:, :].rearrange("(p t) c -> p t c", t=nt)
    out_r = out[:, :].rearrange("(p t) o -> p (t o)", t=nt)

    with TileContext(nc) as tc:
        with (
            tc.tile_pool(name="const", bufs=1) as const_pool,
            tc.tile_pool(name="cb", bufs=1) as cb_pool,
            tc.tile_pool(name="stage", bufs=3) as stage_pool,
            tc.tile_pool(name="bfp", bufs=2) as bfp_pool,
            tc.tile_pool(name="sq", bufs=2) as sq_pool,
            tc.tile_pool(name="lhsT", bufs=2) as lhsT_pool,
            tc.tile_pool(name="cand", bufs=2) as cand_pool,
            tc.tile_pool(name="tp", bufs=2, space="PSUM") as tpsum_pool,
            tc.tile_pool(name="mm", bufs=3, space="PSUM") as mm_pool,
            tc.tile_pool(name="acc", bufs=1) as acc_pool,
            tc.tile_pool(name="fin", bufs=1) as fin_pool,
        ):
            ident = const_pool.tile([P, P], bf16)
            make_identity(nc, ident[:])
            ones2 = const_pool.tile([2, P], bf16)
            nc.vector.memset(ones2[:], 1.0)
            cc2_sb = const_pool.tile([2, M], bf16)
            nc.sync.dma_start(cc2_sb[:], cc2[:, :])

            cbt = []
            for k in range(KC):
                ct = cb_pool.tile([P, M], bf16, tag=f"cb{k}")
                nc.sync.dma_start(ct[:], cbank[k * P:(k + 1) * P, :])
                cbt.append(ct)

            feat = acc_pool.tile([P, nt], f32)
            allv = acc_pool.tile([P, nt * 8], f32)

            def body():
                for t in range(nt):
                    stg = stage_pool.tile([P, C], f32)
                    nc.sync.dma_start(stg[:], phi_r[:, t, :])
                    phib = bfp_pool.tile([P, C], bf16)
                    nc.scalar.copy(phib[:], stg[:])
                    sqt = sq_pool.tile([P, C], bf16)
                    nc.scalar.activation(
                        sqt[:], stg[:], mybir.ActivationFunctionType.Square,
                        accum_out=feat[:, t:t + 1],
                    )

                    tp = tpsum_pool.tile([P, KC * P], bf16)
                    for k in range(KC):
                        nc.tensor.transpose(
                            tp[:, k * P:(k + 1) * P], phib[:, k * P:(k + 1) * P],
                            ident[:],
                        )
                    lt = lhsT_pool.tile([P, KC * P], bf16)
                    nc.vector.tensor_copy(lt[:], tp[:])

                    cand = cand_pool.tile([P, NMT * 8], f32)
                    for j in range(NMT):
                        ps = mm_pool.tile([P, MT], f32)
                        for k in range(KC):
                            nc.tensor.matmul(
                                ps[:],
                                lhsT=lt[:, k * P:(k + 1) * P],
                                rhs=cbt[k][:, j * MT:(j + 1) * MT],
                                start=(k == 0), stop=False,
                            )
                        nc.tensor.matmul(
                            ps[:], lhsT=ones2[:],
                            rhs=cc2_sb[:, j * MT:(j + 1) * MT],
                            start=False, stop=True,
                        )
                        nc.vector.max(out=cand[:, j * 8:(j + 1) * 8], in_=ps[:])
                    nc.vector.max(out=allv[:, t * 8:(t + 1) * 8], in_=cand[:])

                # ---- final: d_i = sqrt(feat - 2*v_i), score = d0/(1+e^g1+e^g2)
                allv_r = allv[:].rearrange("p (t e) -> p e t", e=8)
                d2 = fin_pool.tile([P, 3 * nt], f32)
                for i in range(3):
                    tmp = fin_pool.tile([P, nt], f32, tag=f"tmp{i}")
                    nc.vector.tensor_scalar_mul(tmp[:], allv_r[:, i, :], 2.0)
                    nc.vector.tensor_sub(d2[:, i * nt:(i + 1) * nt], feat[:], tmp[:])
                d = fin_pool.tile([P, 3 * nt], f32)
                nc.scalar.sqrt(d[:], d2[:])
                g = fin_pool.tile([P, 2 * nt], f32)
                nc.vector.tensor_sub(g[:, :nt], d[:, :nt], d[:, nt:2 * nt])
                nc.vector.tensor_sub(g[:, nt:], d[:, :nt], d[:, 2 * nt:])
                e = fin_pool.tile([P, 2 * nt], f32)
                nc.scalar.activation(e[:], g[:], mybir.ActivationFunctionType.Exp)
                s = fin_pool.tile([P, nt], f32)
                nc.vector.tensor_add(s[:], e[:, :nt], e[:, nt:])
                nc.vector.tensor_scalar_add(s[:], s[:], 1.0)
                r = fin_pool.tile([P, nt], f32)
                nc.vector.reciprocal(r[:], s[:])
                sc = fin_pool.tile([P, nt], f32)
                nc.vector.tensor_mul(sc[:], d[:, :nt], r[:])
                nc.sync.dma_start(out_r, sc[:])

            if reps > 1:
                with tc.For_i(0, reps, 1):
                    body()
            else:
                body()

    return nc


def _build_program2(nt=NT, reps=1, korder="kinner", mm_bufs=3, do_max=True, do_feat=True, do_ltdma=True, lt_bufs=3, centers="mm", host_feat=False, stage_bufs=3, cand_bufs=2):
    """v2: phi arrives pre-transposed/bf16 from host (layout prep only);
    no PE transposes, no cast pass, no PSUM-evac copy.
    Row mapping: sbuf row-tile t holds phi rows {p*nt + t}; phit is laid out
    [nt*P, KC*P] with phit[t*128 + p', k*128 + n'] = phi[n'*nt + t, k*128 + p']
    so each tile's lhsT block is one contiguous 448KB DMA (3584B/partition),
    and the output DMA stays contiguous per partition."""
    import concourse.mybir as mybir
    from concourse import bacc
    from concourse.tile import TileContext

    f32 = mybir.dt.float32
    bf16 = mybir.dt.bfloat16
    rows = P * nt

    nc = bacc.Bacc("TRN2", target_bir_lowering=False, debug=False)
    phi = nc.dram_tensor("phi", [rows, C], f32, kind="ExternalInput")
    phit = nc.dram_tensor("phit", [rows, C], bf16, kind="ExternalInput")
    cbank = nc.dram_tensor("cbank", [C, M], bf16, kind="ExternalInput")
    cc2 = nc.dram_tensor("cc2", [2, M], bf16, kind="ExternalInput")
    ccf = (nc.dram_tensor("ccf", [P, M], f32, kind="ExternalInput")
           if centers != "mm" else None)
    featv = (nc.dram_tensor("featv", [P, nt], f32, kind="ExternalInput")
             if host_feat else None)
    out = nc.dram_tensor("out", [rows, 1], f32, kind="ExternalOutput")

    phi_r = phi[:, :].rearrange("(p t) c -> p t c", t=nt)      # feat loads
    phit_r = phit[:, :].rearrange("(t p) f -> t p f", p=P)     # lhsT loads
    out_r = out[:, :].rearrange("(p t) o -> p (t o)", t=nt)

    with TileContext(nc) as tc:
        with (
            tc.tile_pool(name="const", bufs=1) as const_pool,
            tc.tile_pool(name="cb", bufs=1) as cb_pool,
            tc.tile_pool(name="stage", bufs=stage_bufs) as stage_pool,
            tc.tile_pool(name="sq", bufs=2) as sq_pool,
            tc.tile_pool(name="lhsT", bufs=lt_bufs) as lhsT_pool,
            tc.tile_pool(name="cand", bufs=cand_bufs) as cand_pool,
            tc.tile_pool(name="mm", bufs=mm_bufs, space="PSUM") as mm_pool,
            tc.tile_pool(name="mmg", bufs=1, space="PSUM") as mmg_pool,
            tc.tile_pool(name="acc", bufs=1) as acc_pool,
            tc.tile_pool(name="fin", bufs=1) as fin_pool,
        ):
            ones2 = const_pool.tile([2, P], bf16)
            nc.vector.memset(ones2[:], 1.0)
            cc2_sb = const_pool.tile([2, M], bf16)
            nc.sync.dma_start(cc2_sb[:], cc2[:, :])
            ccf_sb = None
            if ccf is not None:
                ccf_sb = const_pool.tile([P, M], f32)
                nc.sync.dma_start(ccf_sb[:], ccf[:, :])

            cbt = []
            for k in range(KC):
                ct = cb_pool.tile([P, M], bf16, tag=f"cb{k}")
                # j=0 slice first so the first matmul group can start after
                # ~1.6MB of C_bank instead of the full 11.2MB
                nc.sync.dma_start(ct[:, 0:MT], cbank[k * P:(k + 1) * P, 0:MT])
                cbt.append(ct)
            for k in range(KC):
                nc.sync.dma_start(cbt[k][:, MT:], cbank[k * P:(k + 1) * P, MT:])

            feat = acc_pool.tile([P, nt], f32)
            allv = acc_pool.tile([P, nt * 8], f32)
            ltfix = None
            if not do_ltdma:
                ltfix = const_pool.tile([P, KC * P], bf16)
                nc.sync.dma_start(ltfix[:], phit_r[0])
            if not do_feat:
                nc.vector.memset(feat[:], 3584.0)
            if host_feat:
                nc.sync.dma_start(feat[:], featv[:, :])

            def body():
                for t in range(nt):
                    if do_feat and not host_feat:
                        stg = stage_pool.tile([P, C], f32)
                        nc.sync.dma_start(stg[:], phi_r[:, t, :])
                        sqt = sq_pool.tile([P, C], bf16)
                        nc.scalar.activation(
                            sqt[:], stg[:], mybir.ActivationFunctionType.Square,
                            accum_out=feat[:, t:t + 1],
                        )
                    if do_ltdma:
                        lt = lhsT_pool.tile([P, KC * P], bf16)
                        nc.sync.dma_start(lt[:], phit_r[t])
                    else:
                        lt = ltfix

                    cand = cand_pool.tile([P, NMT * 8], f32)
                    if korder == "kinner":
                        for j in range(NMT):
                            ps = mm_pool.tile([P, MT], f32)
                            for k in range(KC):
                                nc.tensor.matmul(
                                    ps[:],
                                    lhsT=lt[:, k * P:(k + 1) * P],
                                    rhs=cbt[k][:, j * MT:(j + 1) * MT],
                                    start=(k == 0),
                                    stop=(centers != "mm" and k == KC - 1),
                                )
                            if centers == "mm":
                                nc.tensor.matmul(
                                    ps[:], lhsT=ones2[:],
                                    rhs=cc2_sb[:, j * MT:(j + 1) * MT],
                                    start=False, stop=True,
                                )
                            else:
                                nc.vector.tensor_add(
                                    ps[:], ps[:],
                                    ccf_sb[:, j * MT:(j + 1) * MT],
                                )
                            if do_max:
                                nc.vector.max(out=cand[:, j * 8:(j + 1) * 8],
                                              in_=ps[:])
                    else:  # groups: lhsT constant across consecutive matmuls
                        for grp in ([0, 1, 2], [3, 4, 5, 6]):
                            pss = {j: mmg_pool.tile([P, MT], f32, tag=f"ps{j}",
                                                    name=f"ps{j}_{t}")
                                   for j in grp}
                            for k in range(KC):
                                for j in grp:
                                    nc.tensor.matmul(
                                        pss[j][:],
                                        lhsT=lt[:, k * P:(k + 1) * P],
                                        rhs=cbt[k][:, j * MT:(j + 1) * MT],
                                        start=(k == 0), stop=False,
                                    )
                            for j in grp:
                                nc.tensor.matmul(
                                    pss[j][:], lhsT=ones2[:],
                                    rhs=cc2_sb[:, j * MT:(j + 1) * MT],
                                    start=False, stop=True,
                                )
                            for j in grp:
                                nc.vector.max(out=cand[:, j * 8:(j + 1) * 8],
                                              in_=pss[j][:])
                    if do_max:
                        nc.vector.max(out=allv[:, t * 8:(t + 1) * 8], in_=cand[:])

                if not do_max:
                    nc.sync.dma_start(out_r, feat[:])
                    return
                # ---- final softmin math (same as v1)
                allv_r = allv[:].rearrange("p (t e) -> p e t", e=8)
                d2 = fin_pool.tile([P, 3 * nt], f32)
                for i in range(3):
                    tmp = fin_pool.tile([P, nt], f32, tag=f"tmp{i}")
                    nc.vector.tensor_scalar_mul(tmp[:], allv_r[:, i, :], 2.0)
                    nc.vector.tensor_sub(d2[:, i * nt:(i + 1) * nt], feat[:], tmp[:])
                d = fin_pool.tile([P, 3 * nt], f32)
                nc.scalar.sqrt(d[:], d2[:])
                g = fin_pool.tile([P, 2 * nt], f32)
                nc.vector.tensor_sub(g[:, :nt], d[:, :nt], d[:, nt:2 * nt])
                nc.vector.tensor_sub(g[:, nt:], d[:, :nt], d[:, 2 * nt:])
                e = fin_pool.tile([P, 2 * nt], f32)
                nc.scalar.activation(e[:], g[:], mybir.ActivationFunctionType.Exp)
                s = fin_pool.tile([P, nt], f32)
                nc.vector.tensor_add(s[:], e[:, :nt], e[:, nt:])
                nc.vector.tensor_scalar_add(s[:], s[:], 1.0)
                r = fin_pool.tile([P, nt], f32)
                nc.vector.reciprocal(r[:], s[:])
                sc = fin_pool.tile([P, nt], f32)
                nc.vector.tensor_mul(sc[:], d[:, :nt], r[:])
                nc.sync.dma_start(out_r, sc[:])

            if reps > 1:
                with tc.For_i(0, reps, 1):
                    body()
            else:
                body()

    return nc


def _build_program3(nt=NT, reps=1, groups=(3, 4), lt_bufs=3, mm_bufs=1,
                    cand_bufs=2, centers="add"):
    """v3: fp8 e4m3 DoubleRow matmul, 3 error-compensated passes
    (hi*hi + hi*lo + lo*hi), each DR instruction covering 2 K-tiles at
    0.5 cycles/row -> 4x bf16 MAC rate.  Weight loads amortized by
    streaming `groups` m-tiles per loaded weight tile (j-inner sweeps over
    2 PSUM bank groups so DVE drains overlap the other group's compute).
    |phi|^2 and the -0.5|c|^2 row come precomputed from the host; centers
    row is added on DVE before the max8 selection."""
    import concourse.mybir as mybir
    from concourse import bacc
    from concourse.tile import TileContext

    f32 = mybir.dt.float32
    fp8 = mybir.dt.float8e4
    DR = mybir.MatmulPerfMode.DoubleRow
    rows = P * nt
    KP = KC // 2

    nc = bacc.Bacc("TRN2", target_bir_lowering=False, debug=False)
    phita = nc.dram_tensor("phita", [rows, C], fp8, kind="ExternalInput")
    phitb = nc.dram_tensor("phitb", [rows, C], fp8, kind="ExternalInput")
    cba = nc.dram_tensor("cba", [P, KC * M], fp8, kind="ExternalInput")
    cbb = nc.dram_tensor("cbb", [P, KC * M], fp8, kind="ExternalInput")
    ccf = nc.dram_tensor("ccf", [P, M], f32, kind="ExternalInput")
    featv = nc.dram_tensor("featv", [P, nt], f32, kind="ExternalInput")
    out = nc.dram_tensor("out", [rows, 1], f32, kind="ExternalOutput")

    phita_r = phita[:, :].rearrange("(t p) f -> t p f", p=P)
    phitb_r = phitb[:, :].rearrange("(t p) f -> t p f", p=P)
    out_r = out[:, :].rearrange("(p t) o -> p (t o)", t=nt)

    with TileContext(nc) as tc:
        with (
            tc.tile_pool(name="const", bufs=1) as const_pool,
            tc.tile_pool(name="lta", bufs=lt_bufs) as lta_pool,
            tc.tile_pool(name="ltb", bufs=lt_bufs) as ltb_pool,
            tc.tile_pool(name="cand", bufs=cand_bufs) as cand_pool,
            tc.tile_pool(name="mm", bufs=mm_bufs, space="PSUM") as mm_pool,
            tc.tile_pool(name="acc", bufs=1) as acc_pool,
            tc.tile_pool(name="fin", bufs=1) as fin_pool,
        ):
            cba_sb = const_pool.tile([P, KC * M], fp8)
            cbb_sb = const_pool.tile([P, KC * M], fp8)
            # chunked so the first matmuls only wait on their k-pair block
            for kp in range(KP):
                nc.sync.dma_start(cba_sb[:, 2 * kp * M:(2 * kp + 2) * M],
                                  cba[:, 2 * kp * M:(2 * kp + 2) * M])
            for kp in range(KP):
                nc.sync.dma_start(cbb_sb[:, 2 * kp * M:(2 * kp + 2) * M],
                                  cbb[:, 2 * kp * M:(2 * kp + 2) * M])
            ccf_sb = const_pool.tile([P, M], f32)
            nc.sync.dma_start(ccf_sb[:], ccf[:, :])

            feat = acc_pool.tile([P, nt], f32)
            nc.sync.dma_start(feat[:], featv[:, :])
            allv = acc_pool.tile([P, nt * 8], f32)

            cba_r = cba_sb[:].rearrange("p (k m) -> p k m", k=KC)
            cbb_r = cbb_sb[:].rearrange("p (k m) -> p k m", k=KC)

            def body():
                for t in range(nt):
                    lta = lta_pool.tile([P, C], fp8)
                    nc.sync.dma_start(lta[:], phita_r[t])
                    ltb = ltb_pool.tile([P, C], fp8)
                    nc.sync.dma_start(ltb[:], phitb_r[t])
                    lta_r = lta[:].rearrange("p (k n) -> p k n", k=KC)
                    ltb_r = ltb[:].rearrange("p (k n) -> p k n", k=KC)

                    pss = [mm_pool.tile([P, MT], f32, tag=f"ps{j}",
                                        name=f"ps{j}_{t}")
                           for j in range(NMT)]
                    cand = cand_pool.tile([P, NMT * 8], f32)
                    passes = [(lta_r, cba_r), (lta_r, cbb_r), (ltb_r, cba_r)]
                    if groups == "kinner":
                        for j in range(NMT):
                            for pi, (lt_r, cb_r) in enumerate(passes):
                                for kp in range(KP):
                                    nc.tensor.matmul(
                                        pss[j][:],
                                        lhsT=lt_r[:, 2 * kp:2 * kp + 2, :],
                                        rhs=cb_r[:, 2 * kp:2 * kp + 2,
                                                 j * MT:(j + 1) * MT],
                                        start=(pi == 0 and kp == 0),
                                        stop=(pi == 2 and kp == KP - 1),
                                        perf_mode=DR,
                                    )
                            nc.vector.tensor_add(
                                pss[j][:], pss[j][:],
                                ccf_sb[:, j * MT:(j + 1) * MT])
                            nc.vector.max(out=cand[:, j * 8:(j + 1) * 8],
                                          in_=pss[j][:])
                    else:
                        j0 = 0
                        for gsz in groups:
                            js = list(range(j0, j0 + gsz))
                            j0 += gsz
                            for pi, (lt_r, cb_r) in enumerate(passes):
                                for kp in range(KP):
                                    for j in js:
                                        nc.tensor.matmul(
                                            pss[j][:],
                                            lhsT=lt_r[:, 2 * kp:2 * kp + 2, :],
                                            rhs=cb_r[:, 2 * kp:2 * kp + 2,
                                                     j * MT:(j + 1) * MT],
                                            start=(pi == 0 and kp == 0),
                                            stop=(pi == 2 and kp == KP - 1),
                                            perf_mode=DR,
                                        )
                            for j in js:
                                nc.vector.tensor_add(
                                    pss[j][:], pss[j][:],
                                    ccf_sb[:, j * MT:(j + 1) * MT])
                                nc.vector.max(out=cand[:, j * 8:(j + 1) * 8],
                                              in_=pss[j][:])
                    nc.vector.max(out=allv[:, t * 8:(t + 1) * 8], in_=cand[:])

                # ---- final: d_i = sqrt(feat - 2*v_i), score = d0/(1+e^g1+e^g2)
                allv_r = allv[:].rearrange("p (t e) -> p e t", e=8)
                d2 = fin_pool.tile([P, 3 * nt], f32)
                for i in range(3):
                    tmp = fin_pool.tile([P, nt], f32, tag=f"tmp{i}")
                    nc.vector.tensor_scalar_mul(tmp[:], allv_r[:, i, :], 2.0)
                    nc.vector.tensor_sub(d2[:, i * nt:(i + 1) * nt], feat[:], tmp[:])
                d = fin_pool.tile([P, 3 * nt], f32)
                nc.scalar.sqrt(d[:], d2[:])
                g = fin_pool.tile([P, 2 * nt], f32)
                nc.vector.tensor_sub(g[:, :nt], d[:, :nt], d[:, nt:2 * nt])
                nc.vector.tensor_sub(g[:, nt:], d[:, :nt], d[:, 2 * nt:])
                e = fin_pool.tile([P, 2 * nt], f32)
                nc.scalar.activation(e[:], g[:], mybir.ActivationFunctionType.Exp)
                s = fin_pool.tile([P, nt], f32)
                nc.vector.tensor_add(s[:], e[:, :nt], e[:, nt:])
                nc.vector.tensor_scalar_add(s[:], s[:], 1.0)
                r = fin_pool.tile([P, nt], f32)
                nc.vector.reciprocal(r[:], s[:])
                sc = fin_pool.tile([P, nt], f32)
                nc.vector.tensor_mul(sc[:], d[:, :nt], r[:])
                nc.sync.dma_start(out_r, sc[:])

            if reps > 1:
                with tc.For_i(0, reps, 1):
                    body()
            else:
                body()

    return nc


def _build_program4(nt=NT, reps=1, lt_bufs=3, cand_bufs=2, groups="kinner",
                    mm_rot=None, centers="add"):
    """v4: bf16 single-pass kinner (like v2) but with host-computed |phi|^2
    (no f32 phi load, no Square pass) and the centers row added on DVE
    instead of the 15th K=2 matmul.  PE work = 14 matmuls per (rt, j)."""
    import concourse.mybir as mybir
    from concourse import bacc
    from concourse.tile import TileContext

    f32 = mybir.dt.float32
    bf16 = mybir.dt.bfloat16
    rows = P * nt

    nc = bacc.Bacc("TRN2", target_bir_lowering=False, debug=False)
    phit = nc.dram_tensor("phit", [rows, C], bf16, kind="ExternalInput")
    cbank = nc.dram_tensor("cbank", [C, M], bf16, kind="ExternalInput")
    ccf = nc.dram_tensor("ccf", [P, M], f32, kind="ExternalInput")
    featv = nc.dram_tensor("featv", [P, nt], f32, kind="ExternalInput")
    out = nc.dram_tensor("out", [rows, 1], f32, kind="ExternalOutput")

    phit_r = phit[:, :].rearrange("(t p) f -> t p f", p=P)
    out_r = out[:, :].rearrange("(p t) o -> p (t o)", t=nt)

    with TileContext(nc) as tc:
        with (
            tc.tile_pool(name="const", bufs=1) as const_pool,
            tc.tile_pool(name="cb", bufs=1) as cb_pool,
            tc.tile_pool(name="lhsT", bufs=lt_bufs) as lhsT_pool,
            tc.tile_pool(name="cand", bufs=cand_bufs) as cand_pool,
            tc.tile_pool(name="mm",
                         bufs=(6 if groups == "kinner" else (mm_rot or 1)),
                         space="PSUM") as mm_pool,
            tc.tile_pool(name="acc", bufs=1) as acc_pool,
            tc.tile_pool(name="fin", bufs=1) as fin_pool,
        ):
            ccf_sb = const_pool.tile([P, M], f32)
            nc.sync.dma_start(ccf_sb[:], ccf[:, :])
            cbt = []
            for k in range(KC):
                ct = cb_pool.tile([P, M], bf16, tag=f"cb{k}")
                nc.sync.dma_start(ct[:, 0:MT], cbank[k * P:(k + 1) * P, 0:MT])
                cbt.append(ct)
            for k in range(KC):
                nc.sync.dma_start(cbt[k][:, MT:], cbank[k * P:(k + 1) * P, MT:])

            feat = acc_pool.tile([P, nt], f32)
            nc.sync.dma_start(feat[:], featv[:, :])
            allv = acc_pool.tile([P, nt * 8], f32)

            def body():
                for t in range(nt):
                    lt = lhsT_pool.tile([P, KC * P], bf16)
                    nc.sync.dma_start(lt[:], phit_r[t])
                    cand = cand_pool.tile([P, NMT * 8], f32)
                    if groups == "kinner":
                        for j in range(NMT):
                            ps = mm_pool.tile([P, MT], f32)
                            for k in range(KC):
                                nc.tensor.matmul(
                                    ps[:],
                                    lhsT=lt[:, k * P:(k + 1) * P],
                                    rhs=cbt[k][:, j * MT:(j + 1) * MT],
                                    start=(k == 0), stop=(k == KC - 1),
                                )
                            nc.vector.tensor_add(
                                ps[:], ps[:], ccf_sb[:, j * MT:(j + 1) * MT])
                            nc.vector.max(out=cand[:, j * 8:(j + 1) * 8],
                                          in_=ps[:])
                    else:
                        pss = [mm_pool.tile([P, MT], f32,
                                            tag=("psr" if mm_rot
                                                 else f"ps{j}"),
                                            name=f"ps{j}_{t}")
                               for j in range(NMT)]
                        j0 = 0
                        for gsz in groups:
                            js = list(range(j0, j0 + gsz))
                            j0 += gsz
                            if centers == "preload":
                                # seed psum with the centers row; matmuls
                                # accumulate on top (start=False), so the
                                # drain is just the max8
                                for j in js:
                                    nc.vector.tensor_copy(
                                        pss[j][:],
                                        ccf_sb[:, j * MT:(j + 1) * MT])
                            for k in range(KC):
                                for j in js:
                                    nc.tensor.matmul(
                                        pss[j][:],
                                        lhsT=lt[:, k * P:(k + 1) * P],
                                        rhs=cbt[k][:, j * MT:(j + 1) * MT],
                                        start=(k == 0 and centers != "preload"),
                                        stop=(k == KC - 1),
                                        skip_group_check=(centers == "preload"),
                                    )
                            for j in js:
                                if centers != "preload":
                                    nc.vector.tensor_add(
                                        pss[j][:], pss[j][:],
                                        ccf_sb[:, j * MT:(j + 1) * MT])
                                nc.vector.max(out=cand[:, j * 8:(j + 1) * 8],
                                              in_=pss[j][:])
                    nc.vector.max(out=allv[:, t * 8:(t + 1) * 8], in_=cand[:])

                # ---- final softmin math
                allv_r = allv[:].rearrange("p (t e) -> p e t", e=8)
                d2 = fin_pool.tile([P, 3 * nt], f32)
                for i in range(3):
                    tmp = fin_pool.tile([P, nt], f32, tag=f"tmp{i}")
                    nc.vector.tensor_scalar_mul(tmp[:], allv_r[:, i, :], 2.0)
                    nc.vector.tensor_sub(d2[:, i * nt:(i + 1) * nt], feat[:], tmp[:])
                d = fin_pool.tile([P, 3 * nt], f32)
                nc.scalar.sqrt(d[:], d2[:])
                g = fin_pool.tile([P, 2 * nt], f32)
                nc.vector.tensor_sub(g[:, :nt], d[:, :nt], d[:, nt:2 * nt])
                nc.vector.tensor_sub(g[:, nt:], d[:, :nt], d[:, 2 * nt:])
                e = fin_pool.tile([P, 2 * nt], f32)
                nc.scalar.activation(e[:], g[:], mybir.ActivationFunctionType.Exp)
                s = fin_pool.tile([P, nt], f32)
                nc.vector.tensor_add(s[:], e[:, :nt], e[:, nt:])
                nc.vector.tensor_scalar_add(s[:], s[:], 1.0)
                r = fin_pool.tile([P, nt], f32)
                nc.vector.reciprocal(r[:], s[:])
                sc = fin_pool.tile([P, nt], f32)
                nc.vector.tensor_mul(sc[:], d[:, :nt], r[:])
                nc.sync.dma_start(out_r, sc[:])

            if reps > 1:
                with tc.For_i(0, reps, 1):
                    body()
            else:
                body()

    return nc


def _make_in_maps4(phi_p, C_bank):
    import ml_dtypes
    cb_bf = np.ascontiguousarray(C_bank.astype(ml_dtypes.bfloat16))
    row = -0.5 * (C_bank.astype(np.float64) ** 2).sum(0)
    ccf = np.ascontiguousarray(
        np.broadcast_to(row.astype(np.float32), (P, M)))
    phi2 = np.ascontiguousarray(phi_p.reshape(B * HW, C))
    in_maps = []
    for k in range(NCORES):
        pc = phi2[k * ROWS:(k + 1) * ROWS]
        in_maps.append({"phit": _host_prep_phit(pc), "cbank": cb_bf,
                        "ccf": ccf, "featv": _host_feat(pc)})
    return in_maps


def _host_prep_phit(phi_core, nt=NT):
    """[rows, C] f32 -> [nt*P, KC*P] bf16, laid out so lhsT tile t is one
    contiguous 448KB block: phit[t*128 + p', k*128 + n'] = phi[t*128 + n', k*128 + p']."""
    import ml_dtypes
    # tile t, sbuf partition p' (= contraction c_local), free n' (= within-tile
    # row index); within-tile row n' maps to phi row n'*nt + t (v1 mapping).
    x = phi_core.reshape(P, nt, KC, P).transpose(1, 3, 2, 0)   # [t, p', k, n']
    return np.ascontiguousarray(x.reshape(nt * P, KC * P).astype(ml_dtypes.bfloat16))


def _host_prep(C_bank):
    import ml_dtypes
    bf = ml_dtypes.bfloat16
    cb_bf = np.ascontiguousarray(C_bank.astype(bf))
    row = -0.5 * (C_bank.astype(np.float64) ** 2).sum(0)
    chi = row.astype(np.float32).astype(bf)
    clo = (row - chi.astype(np.float64)).astype(np.float32).astype(bf)
    cc2 = np.ascontiguousarray(np.stack([chi, clo]))
    ccf = np.ascontiguousarray(
        np.broadcast_to(row.astype(np.float32), (P, C_bank.shape[1])))
    return cb_bf, cc2, ccf


def _host_prep_cb8(C_bank):
    """C_bank [C, M] f32 -> (cba, cbb) fp8 e4m3 hi/lo in k-major SBUF layout
    [P, KC*M] with cb[p, k*M + m] = x[k*128 + p, m], plus ccf [P, M] f32
    broadcast of the -0.5|c_m|^2 row."""
    import ml_dtypes
    e4 = ml_dtypes.float8_e4m3
    hi = C_bank.astype(e4)
    lo = (C_bank - hi.astype(np.float32)).astype(e4)

    def lay(x):
        return np.ascontiguousarray(
            x.reshape(KC, P, M).transpose(1, 0, 2).reshape(P, KC * M))

    row = -0.5 * (C_bank.astype(np.float64) ** 2).sum(0)
    ccf = np.ascontiguousarray(
        np.broadcast_to(row.astype(np.float32), (P, M)))
    return lay(hi), lay(lo), ccf


def _host_prep_phit8(phi_core, nt=NT):
    """[rows, C] f32 -> (hi, lo) fp8 e4m3 in the transposed lhsT layout
    phit[t*128 + p', k*128 + n'] = phi[p... row n'*nt + t, k*128 + p']."""
    import ml_dtypes
    e4 = ml_dtypes.float8_e4m3
    x = phi_core.reshape(P, nt, KC, P).transpose(1, 3, 2, 0)
    x = np.ascontiguousarray(x.reshape(nt * P, KC * P))
    hi = x.astype(e4)
    lo = (x - hi.astype(np.float32)).astype(e4)
    return hi, lo


def _host_feat(phi_core, nt=NT):
    """[rows, C] f32 -> [P, nt] f32 of |phi_row|^2, row = p*nt + t."""
    return np.einsum("rc,rc->r", phi_core, phi_core).reshape(P, nt)


def _make_in_maps3(phi_p, C_bank):
    cba, cbb, ccf = _host_prep_cb8(C_bank)
    phi2 = np.ascontiguousarray(phi_p.reshape(B * HW, C))
    in_maps = []
    for k in range(NCORES):
        pc = phi2[k * ROWS:(k + 1) * ROWS]
        pa, pb = _host_prep_phit8(pc)
        in_maps.append({"phita": pa, "phitb": pb, "cba": cba, "cbb": cbb,
                        "ccf": ccf, "featv": _host_feat(pc)})
    return in_maps


def kernel(phi_p: np.ndarray, C_bank: np.ndarray) -> np.ndarray:
    from concourse.bass_utils import run_bass_kernel_spmd

    if "nc" not in _CACHE:
        nc = _build_program4(**KERNEL_KW)
        nc.finalize()
        _CACHE["nc"] = nc
    nc = _CACHE["nc"]

    phi_p = np.asarray(phi_p, dtype=np.float32)
    C_bank = np.asarray(C_bank, dtype=np.float32)
    in_maps = _make_in_maps4(phi_p, C_bank)
    res = None
    for attempt in range(3):
        try:
            res = run_bass_kernel_spmd(nc, in_maps, list(range(NCORES)))
            break
        except Exception:
            # transient NRT device errors have been observed; reset the jax
            # backend connection and retry
            if attempt == 2:
                raise
            import time as _time
            _time.sleep(5)
            try:
                import jax
                jax.clear_caches()
                jax.extend.backend.clear_backends()
            except Exception:
                pass
    out = np.concatenate([res.results[k]["out"] for k in range(NCORES)], axis=0)
    return out.reshape(B, HW, 1)



# revision 7
# speedup vs baseline: 1.2419x; 1.2419x over previous
"""Trainium2 kernel for CFA-style KNN retrieval scoring.

Computes, for each row of phi [B*HW, C]:
  d_m = sqrt(|phi|^2 + |c_m|^2 - 2 phi.c_m)  over M bank entries,
  top-3 smallest distances d0<=d1<=d2,
  score = d0 / (1 + exp(d0-d1) + exp(d0-d2))      (= softmin weight * d0)

Strategy (8 NeuronCores, data-parallel over rows):
 - shard rows (B*HW = 50176) into 8 contiguous chunks of 6272 rows
 - bf16 matmul on TensorE (fp32 PSUM accumulate); the -0.5*|c_m|^2 term is
   folded into the accumulation via a K=2 ones-matmul with a bf16 hi/lo
   split of the centers row (keeps its error ~1e-3 instead of bf16 ulp ~4)
 - selection runs on psum = phi.c - 0.5|c|^2 directly (|phi|^2 is constant
   per row, monotone under sqrt -> does not affect the ranking); DVE `max`
   (top-8) per 448-wide PSUM tile, then a second `max` over 56 candidates
 - |phi|^2 computed in fp32 on ScalarE (activation Square + accum)
 - final softmin math on 3 values/row at the end, batched over all tiles
"""

import sys

import numpy as np

if "/opt/trn_rl_repo" not in sys.path:
    sys.path.insert(0, "/opt/trn_rl_repo")

B, HW, C, M = 16, 3136, 1792, 3136
NCORES = 8
ROWS = B * HW // NCORES     # 6272 rows per core
P = 128                     # partitions
NT = ROWS // P              # 49 row-tiles per core; row = p*NT + t
KC = C // P                 # 14 contraction chunks
MT = 448                    # matmul moving free size (one PSUM bank)
NMT = M // MT               # 7 m-tiles

_CACHE = {}
# selected by benchmarking; see _build_program4
KERNEL_KW = dict(groups=(3, 4))
KERNEL_GROUPS = (3, 4)   # kept for compat with older test.py paths


def _build_program(nt=NT, reps=1):
    import contextlib
    import concourse.mybir as mybir
    from concourse import bacc
    from concourse.tile import TileContext
    from concourse.masks import make_identity

    f32 = mybir.dt.float32
    bf16 = mybir.dt.bfloat16
    rows = P * nt

    nc = bacc.Bacc("TRN2", target_bir_lowering=False, debug=False)
    phi = nc.dram_tensor("phi", [rows, C], f32, kind="ExternalInput")
    cbank = nc.dram_tensor("cbank", [C, M], bf16, kind="ExternalInput")
    cc2 = nc.dram_tensor("cc2", [2, M], bf16, kind="ExternalInput")
    out = nc.dram_tensor("out", [rows, 1], f32, kind="ExternalOutput")

    phi_r = phi[:, :].rearrange("(p t) c -> p t c", t=nt)
    out_r = out[:, :].rearrange("(p t) o -> p (t o)", t=nt)

    with TileContext(nc) as tc:
        with (
            tc.tile_pool(name="const", bufs=1) as const_pool,
            tc.tile_pool(name="cb", bufs=1) as cb_pool,
            tc.tile_pool(name="stage", bufs=3) as stage_pool,
            tc.tile_pool(name="bfp", bufs=2) as bfp_pool,
            tc.tile_pool(name="sq", bufs=2) as sq_pool,
            tc.tile_pool(name="lhsT", bufs=2) as lhsT_pool,
            tc.tile_pool(name="cand", bufs=2) as cand_pool,
            tc.tile_pool(name="tp", bufs=2, space="PSUM") as tpsum_pool,
            tc.tile_pool(name="mm", bufs=3, space="PSUM") as mm_pool,
            tc.tile_pool(name="acc", bufs=1) as acc_pool,
            tc.tile_pool(name="fin", bufs=1) as fin_pool,
        ):
            ident = const_pool.tile([P, P], bf16)
            make_identity(nc, ident[:])
            ones2 = const_pool.tile([2, P], bf16)
            nc.vector.memset(ones2[:], 1.0)
            cc2_sb = const_pool.tile([2, M], bf16)
            nc.sync.dma_start(cc2_sb[:], cc2[:, :])

            cbt = []
            for k in range(KC):
                ct = cb_pool.tile([P, M], bf16, tag=f"cb{k}")
                nc.sync.dma_start(ct[:], cbank[k * P:(k + 1) * P, :])
                cbt.append(ct)

            feat = acc_pool.tile([P, nt], f32)
            allv = acc_pool.tile([P, nt * 8], f32)

            def body():
                for t in range(nt):
                    stg = stage_pool.tile([P, C], f32)
                    nc.sync.dma_start(stg[:], phi_r[:, t, :])
                    phib = bfp_pool.tile([P, C], bf16)
                    nc.scalar.copy(phib[:], stg[:])
                    sqt = sq_pool.tile([P, C], bf16)
                    nc.scalar.activation(
                        sqt[:], stg[:], mybir.ActivationFunctionType.Square,
                        accum_out=feat[:, t:t + 1],
                    )

                    tp = tpsum_pool.tile([P, KC * P], bf16)
                    for k in range(KC):
                        nc.tensor.transpose(
                            tp[:, k * P:(k + 1) * P], phib[:, k * P:(k + 1) * P],
                            ident[:],
                        )
                    lt = lhsT_pool.tile([P, KC * P], bf16)
                    nc.vector.tensor_copy(lt[:], tp[:])

                    cand = cand_pool.tile([P, NMT * 8], f32)
                    for j in range(NMT):
                        ps = mm_pool.tile([P, MT], f32)
                        for k in range(KC):
                            nc.tensor.matmul(
                                ps[:],
                                lhsT=lt[:, k * P:(k + 1) * P],
                                rhs=cbt[k][:, j * MT:(j + 1) * MT],
                                start=(k == 0), stop=False,
                            )
                        nc.tensor.matmul(
                            ps[:], lhsT=ones2[:],
                            rhs=cc2_sb[:, j * MT:(j + 1) * MT],
                            start=False, stop=True,
                        )
                        nc.vector.max(out=cand[:, j * 8:(j + 1) * 8], in_=ps[:])
                    nc.vector.max(out=allv[:, t * 8:(t + 1) * 8], in_=cand[:])

                # ---- final: d_i = sqrt(feat - 2*v_i), score = d0/(1+e^g1+e^g2)
                allv_r = allv[:].rearrange("p (t e) -> p e t", e=8)
                d2 = fin_pool.tile([P, 3 * nt], f32)
                for i in range(3):
                    tmp = fin_pool.tile([P, nt], f32, tag=f"tmp{i}")
                    nc.vector.tensor_scalar_mul(tmp[:], allv_r[:, i, :], 2.0)
                    nc.vector.tensor_sub(d2[:, i * nt:(i + 1) * nt], feat[:], tmp[:])
                d = fin_pool.tile([P, 3 * nt], f32)
                nc.scalar.sqrt(d[:], d2[:])
                g = fin_pool.tile([P, 2 * nt], f32)
                nc.vector.tensor_sub(g[:, :nt], d[:, :nt], d[:, nt:2 * nt])
                nc.vector.tensor_sub(g[:, nt:], d[:, :nt], d[:, 2 * nt:])
                e = fin_pool.tile([P, 2 * nt], f32)
                nc.scalar.activation(e[:], g[:], mybir.ActivationFunctionType.Exp)
                s = fin_pool.tile([P, nt], f32)
                nc.vector.tensor_add(s[:], e[:, :nt], e[:, nt:])
                nc.vector.tensor_scalar_add(s[:], s[:], 1.0)
                r = fin_pool.tile([P, nt], f32)
                nc.vector.reciprocal(r[:], s[:])
                sc = fin_pool.tile([P, nt], f32)
                nc.vector.tensor_mul(sc[:], d[:, :nt], r[:])
                nc.sync.dma_start(out_r, sc[:])

            if reps > 1:
                with tc.For_i(0, reps, 1):
                    body()
            else:
                body()

    return nc


def _build_program2(nt=NT, reps=1, korder="kinner", mm_bufs=3, do_max=True, do_feat=True, do_ltdma=True, lt_bufs=3, centers="mm", host_feat=False, stage_bufs=3, cand_bufs=2):
    """v2: phi arrives pre-transposed/bf16 from host (layout prep only);
    no PE transposes, no cast pass, no PSUM-evac copy.
    Row mapping: sbuf row-tile t holds phi rows {p*nt + t}; phit is laid out
    [nt*P, KC*P] with phit[t*128 + p', k*128 + n'] = phi[n'*nt + t, k*128 + p']
    so each tile's lhsT block is one contiguous 448KB DMA (3584B/partition),
    and the output DMA stays contiguous per partition."""
    import concourse.mybir as mybir
    from concourse import bacc
    from concourse.tile import TileContext

    f32 = mybir.dt.float32
    bf16 = mybir.dt.bfloat16
    rows = P * nt

    nc = bacc.Bacc("TRN2", target_bir_lowering=False, debug=False)
    phi = nc.dram_tensor("phi", [rows, C], f32, kind="ExternalInput")
    phit = nc.dram_tensor("phit", [rows, C], bf16, kind="ExternalInput")
    cbank = nc.dram_tensor("cbank", [C, M], bf16, kind="ExternalInput")
    cc2 = nc.dram_tensor("cc2", [2, M], bf16, kind="ExternalInput")
    ccf = (nc.dram_tensor("ccf", [P, M], f32, kind="ExternalInput")
           if centers != "mm" else None)
    featv = (nc.dram_tensor("featv", [P, nt], f32, kind="ExternalInput")
             if host_feat else None)
    out = nc.dram_tensor("out", [rows, 1], f32, kind="ExternalOutput")

    phi_r = phi[:, :].rearrange("(p t) c -> p t c", t=nt)      # feat loads
    phit_r = phit[:, :].rearrange("(t p) f -> t p f", p=P)     # lhsT loads
    out_r = out[:, :].rearrange("(p t) o -> p (t o)", t=nt)

    with TileContext(nc) as tc:
        with (
            tc.tile_pool(name="const", bufs=1) as const_pool,
            tc.tile_pool(name="cb", bufs=1) as cb_pool,
            tc.tile_pool(name="stage", bufs=stage_bufs) as stage_pool,
            tc.tile_pool(name="sq", bufs=2) as sq_pool,
            tc.tile_pool(name="lhsT", bufs=lt_bufs) as lhsT_pool,
            tc.tile_pool(name="cand", bufs=cand_bufs) as cand_pool,
            tc.tile_pool(name="mm", bufs=mm_bufs, space="PSUM") as mm_pool,
            tc.tile_pool(name="mmg", bufs=1, space="PSUM") as mmg_pool,
            tc.tile_pool(name="acc", bufs=1) as acc_pool,
            tc.tile_pool(name="fin", bufs=1) as fin_pool,
        ):
            ones2 = const_pool.tile([2, P], bf16)
            nc.vector.memset(ones2[:], 1.0)
            cc2_sb = const_pool.tile([2, M], bf16)
            nc.sync.dma_start(cc2_sb[:], cc2[:, :])
            ccf_sb = None
            if ccf is not None:
                ccf_sb = const_pool.tile([P, M], f32)
                nc.sync.dma_start(ccf_sb[:], ccf[:, :])

            cbt = []
            for k in range(KC):
                ct = cb_pool.tile([P, M], bf16, tag=f"cb{k}")
                # j=0 slice first so the first matmul group can start after
                # ~1.6MB of C_bank instead of the full 11.2MB
                nc.sync.dma_start(ct[:, 0:MT], cbank[k * P:(k + 1) * P, 0:MT])
                cbt.append(ct)
            for k in range(KC):
                nc.sync.dma_start(cbt[k][:, MT:], cbank[k * P:(k + 1) * P, MT:])

            feat = acc_pool.tile([P, nt], f32)
            allv = acc_pool.tile([P, nt * 8], f32)
            ltfix = None
            if not do_ltdma:
                ltfix = const_pool.tile([P, KC * P], bf16)
                nc.sync.dma_start(ltfix[:], phit_r[0])
            if not do_feat:
                nc.vector.memset(feat[:], 3584.0)
            if host_feat:
                nc.sync.dma_start(feat[:], featv[:, :])

            def body():
                for t in range(nt):
                    if do_feat and not host_feat:
                        stg = stage_pool.tile([P, C], f32)
                        nc.sync.dma_start(stg[:], phi_r[:, t, :])
                        sqt = sq_pool.tile([P, C], bf16)
                        nc.scalar.activation(
                            sqt[:], stg[:], mybir.ActivationFunctionType.Square,
                            accum_out=feat[:, t:t + 1],
                        )
                    if do_ltdma:
                        lt = lhsT_pool.tile([P, KC * P], bf16)
                        nc.sync.dma_start(lt[:], phit_r[t])
                    else:
                        lt = ltfix

                    cand = cand_pool.tile([P, NMT * 8], f32)
                    if korder == "kinner":
                        for j in range(NMT):
                            ps = mm_pool.tile([P, MT], f32)
                            for k in range(KC):
                                nc.tensor.matmul(
                                    ps[:],
                                    lhsT=lt[:, k * P:(k + 1) * P],
                                    rhs=cbt[k][:, j * MT:(j + 1) * MT],
                                    start=(k == 0),
                                    stop=(centers != "mm" and k == KC - 1),
                                )
                            if centers == "mm":
                                nc.tensor.matmul(
                                    ps[:], lhsT=ones2[:],
                                    rhs=cc2_sb[:, j * MT:(j + 1) * MT],
                                    start=False, stop=True,
                                )
                            else:
                                nc.vector.tensor_add(
                                    ps[:], ps[:],
                                    ccf_sb[:, j * MT:(j + 1) * MT],
                                )
                            if do_max:
                                nc.vector.max(out=cand[:, j * 8:(j + 1) * 8],
                                              in_=ps[:])
                    else:  # groups: lhsT constant across consecutive matmuls
                        for grp in ([0, 1, 2], [3, 4, 5, 6]):
                            pss = {j: mmg_pool.tile([P, MT], f32, tag=f"ps{j}",
                                                    name=f"ps{j}_{t}")
                                   for j in grp}
                            for k in range(KC):
                                for j in grp:
                                    nc.tensor.matmul(
                                        pss[j][:],
                                        lhsT=lt[:, k * P:(k + 1) * P],
                                        rhs=cbt[k][:, j * MT:(j + 1) * MT],
                                        start=(k == 0), stop=False,
                                    )
                            for j in grp:
                                nc.tensor.matmul(
                                    pss[j][:], lhsT=ones2[:],
                                    rhs=cc2_sb[:, j * MT:(j + 1) * MT],
                                    start=False, stop=True,
                                )
                            for j in grp:
                                nc.vector.max(out=cand[:, j * 8:(j + 1) * 8],
                                              in_=pss[j][:])
                    if do_max:
                        nc.vector.max(out=allv[:, t * 8:(t + 1) * 8], in_=cand[:])

                if not do_max:
                    nc.sync.dma_start(out_r, feat[:])
                    return
                # ---- final softmin math (same as v1)
                allv_r = allv[:].rearrange("p (t e) -> p e t", e=8)
                d2 = fin_pool.tile([P, 3 * nt], f32)
                for i in range(3):
                    tmp = fin_pool.tile([P, nt], f32, tag=f"tmp{i}")
                    nc.vector.tensor_scalar_mul(tmp[:], allv_r[:, i, :], 2.0)
                    nc.vector.tensor_sub(d2[:, i * nt:(i + 1) * nt], feat[:], tmp[:])
                d = fin_pool.tile([P, 3 * nt], f32)
                nc.scalar.sqrt(d[:], d2[:])
                g = fin_pool.tile([P, 2 * nt], f32)
                nc.vector.tensor_sub(g[:, :nt], d[:, :nt], d[:, nt:2 * nt])
                nc.vector.tensor_sub(g[:, nt:], d[:, :nt], d[:, 2 * nt:])
                e = fin_pool.tile([P, 2 * nt], f32)
                nc.scalar.activation(e[:], g[:], mybir.ActivationFunctionType.Exp)
                s = fin_pool.tile([P, nt], f32)
                nc.vector.tensor_add(s[:], e[:, :nt], e[:, nt:])
                nc.vector.tensor_scalar_add(s[:], s[:], 1.0)
                r = fin_pool.tile([P, nt], f32)
                nc.vector.reciprocal(r[:], s[:])
                sc = fin_pool.tile([P, nt], f32)
                nc.vector.tensor_mul(sc[:], d[:, :nt], r[:])
                nc.sync.dma_start(out_r, sc[:])

            if reps > 1:
                with tc.For_i(0, reps, 1):
                    body()
            else:
                body()

    return nc


def _build_program3(nt=NT, reps=1, groups=(3, 4), lt_bufs=3, mm_bufs=1,
                    cand_bufs=2, centers="add", npass=3):
    """v3: fp8 e4m3 DoubleRow matmul, 3 error-compensated passes
    (hi*hi + hi*lo + lo*hi), each DR instruction covering 2 K-tiles at
    0.5 cycles/row -> 4x bf16 MAC rate.  Weight loads amortized by
    streaming `groups` m-tiles per loaded weight tile (j-inner sweeps over
    2 PSUM bank groups so DVE drains overlap the other group's compute).
    |phi|^2 and the -0.5|c|^2 row come precomputed from the host; centers
    row is added on DVE before the max8 selection."""
    import concourse.mybir as mybir
    from concourse import bacc
    from concourse.tile import TileContext

    f32 = mybir.dt.float32
    fp8 = mybir.dt.float8e4
    DR = mybir.MatmulPerfMode.DoubleRow
    rows = P * nt
    KP = KC // 2

    nc = bacc.Bacc("TRN2", target_bir_lowering=False, debug=False)
    phita = nc.dram_tensor("phita", [rows, C], fp8, kind="ExternalInput")
    phitb = nc.dram_tensor("phitb", [rows, C], fp8, kind="ExternalInput")
    cba = nc.dram_tensor("cba", [P, KC * M], fp8, kind="ExternalInput")
    cbb = nc.dram_tensor("cbb", [P, KC * M], fp8, kind="ExternalInput")
    ccf = nc.dram_tensor("ccf", [P, M], f32, kind="ExternalInput")
    featv = nc.dram_tensor("featv", [P, nt], f32, kind="ExternalInput")
    out = nc.dram_tensor("out", [rows, 1], f32, kind="ExternalOutput")

    phita_r = phita[:, :].rearrange("(t p) f -> t p f", p=P)
    phitb_r = phitb[:, :].rearrange("(t p) f -> t p f", p=P)
    out_r = out[:, :].rearrange("(p t) o -> p (t o)", t=nt)

    with TileContext(nc) as tc:
        with (
            tc.tile_pool(name="const", bufs=1) as const_pool,
            tc.tile_pool(name="lta", bufs=lt_bufs) as lta_pool,
            tc.tile_pool(name="ltb", bufs=lt_bufs) as ltb_pool,
            tc.tile_pool(name="cand", bufs=cand_bufs) as cand_pool,
            tc.tile_pool(name="mm", bufs=mm_bufs, space="PSUM") as mm_pool,
            tc.tile_pool(name="acc", bufs=1) as acc_pool,
            tc.tile_pool(name="fin", bufs=1) as fin_pool,
        ):
            cba_sb = const_pool.tile([P, KC * M], fp8)
            cbb_sb = const_pool.tile([P, KC * M], fp8)
            # chunked so the first matmuls only wait on their k-pair block
            for kp in range(KP):
                nc.sync.dma_start(cba_sb[:, 2 * kp * M:(2 * kp + 2) * M],
                                  cba[:, 2 * kp * M:(2 * kp + 2) * M])
            for kp in range(KP):
                nc.sync.dma_start(cbb_sb[:, 2 * kp * M:(2 * kp + 2) * M],
                                  cbb[:, 2 * kp * M:(2 * kp + 2) * M])
            ccf_sb = const_pool.tile([P, M], f32)
            nc.sync.dma_start(ccf_sb[:], ccf[:, :])

            feat = acc_pool.tile([P, nt], f32)
            nc.sync.dma_start(feat[:], featv[:, :])
            allv = acc_pool.tile([P, nt * 8], f32)

            cba_r = cba_sb[:].rearrange("p (k m) -> p k m", k=KC)
            cbb_r = cbb_sb[:].rearrange("p (k m) -> p k m", k=KC)

            def body():
                for t in range(nt):
                    lta = lta_pool.tile([P, C], fp8)
                    nc.sync.dma_start(lta[:], phita_r[t])
                    ltb = ltb_pool.tile([P, C], fp8)
                    nc.sync.dma_start(ltb[:], phitb_r[t])
                    lta_r = lta[:].rearrange("p (k n) -> p k n", k=KC)
                    ltb_r = ltb[:].rearrange("p (k n) -> p k n", k=KC)

                    pss = [mm_pool.tile([P, MT], f32, tag=f"ps{j}",
                                        name=f"ps{j}_{t}")
                           for j in range(NMT)]
                    cand = cand_pool.tile([P, NMT * 8], f32)
                    passes = [(lta_r, cba_r), (lta_r, cbb_r),
                              (ltb_r, cba_r)][:npass]
                    if groups == "kinner":
                        for j in range(NMT):
                            for pi, (lt_r, cb_r) in enumerate(passes):
                                for kp in range(KP):
                                    nc.tensor.matmul(
                                        pss[j][:],
                                        lhsT=lt_r[:, 2 * kp:2 * kp + 2, :],
                                        rhs=cb_r[:, 2 * kp:2 * kp + 2,
                                                 j * MT:(j + 1) * MT],
                                        start=(pi == 0 and kp == 0),
                                        stop=(pi == npass - 1 and kp == KP - 1),
                                        perf_mode=DR,
                                    )
                            nc.vector.tensor_add(
                                pss[j][:], pss[j][:],
                                ccf_sb[:, j * MT:(j + 1) * MT])
                            nc.vector.max(out=cand[:, j * 8:(j + 1) * 8],
                                          in_=pss[j][:])
                    else:
                        j0 = 0
                        for gsz in groups:
                            js = list(range(j0, j0 + gsz))
                            j0 += gsz
                            for pi, (lt_r, cb_r) in enumerate(passes):
                                for kp in range(KP):
                                    for j in js:
                                        nc.tensor.matmul(
                                            pss[j][:],
                                            lhsT=lt_r[:, 2 * kp:2 * kp + 2, :],
                                            rhs=cb_r[:, 2 * kp:2 * kp + 2,
                                                     j * MT:(j + 1) * MT],
                                            start=(pi == 0 and kp == 0),
                                            stop=(pi == npass - 1 and kp == KP - 1),
                                            perf_mode=DR,
                                        )
                            for j in js:
                                nc.vector.tensor_add(
                                    pss[j][:], pss[j][:],
                                    ccf_sb[:, j * MT:(j + 1) * MT])
                                nc.vector.max(out=cand[:, j * 8:(j + 1) * 8],
                                              in_=pss[j][:])
                    nc.vector.max(out=allv[:, t * 8:(t + 1) * 8], in_=cand[:])

                # ---- final: d_i = sqrt(feat - 2*v_i), score = d0/(1+e^g1+e^g2)
                allv_r = allv[:].rearrange("p (t e) -> p e t", e=8)
                d2 = fin_pool.tile([P, 3 * nt], f32)
                for i in range(3):
                    tmp = fin_pool.tile([P, nt], f32, tag=f"tmp{i}")
                    nc.vector.tensor_scalar_mul(tmp[:], allv_r[:, i, :], 2.0)
                    nc.vector.tensor_sub(d2[:, i * nt:(i + 1) * nt], feat[:], tmp[:])
                d = fin_pool.tile([P, 3 * nt], f32)
                nc.scalar.sqrt(d[:], d2[:])
                g = fin_pool.tile([P, 2 * nt], f32)
                nc.vector.tensor_sub(g[:, :nt], d[:, :nt], d[:, nt:2 * nt])
                nc.vector.tensor_sub(g[:, nt:], d[:, :nt], d[:, 2 * nt:])
                e = fin_pool.tile([P, 2 * nt], f32)
                nc.scalar.activation(e[:], g[:], mybir.ActivationFunctionType.Exp)
                s = fin_pool.tile([P, nt], f32)
                nc.vector.tensor_add(s[:], e[:, :nt], e[:, nt:])
                nc.vector.tensor_scalar_add(s[:], s[:], 1.0)
                r = fin_pool.tile([P, nt], f32)
                nc.vector.reciprocal(r[:], s[:])
                sc = fin_pool.tile([P, nt], f32)
                nc.vector.tensor_mul(sc[:], d[:, :nt], r[:])
                nc.sync.dma_start(out_r, sc[:])

            if reps > 1:
                with tc.For_i(0, reps, 1):
                    body()
            else:
                body()

    return nc


def _build_program4(nt=NT, reps=1, lt_bufs=3, cand_bufs=2, groups="kinner",
                    mm_rot=None, centers="add"):
    """v4: bf16 single-pass kinner (like v2) but with host-computed |phi|^2
    (no f32 phi load, no Square pass) and the centers row added on DVE
    instead of the 15th K=2 matmul.  PE work = 14 matmuls per (rt, j)."""
    import concourse.mybir as mybir
    from concourse import bacc
    from concourse.tile import TileContext

    f32 = mybir.dt.float32
    bf16 = mybir.dt.bfloat16
    rows = P * nt

    nc = bacc.Bacc("TRN2", target_bir_lowering=False, debug=False)
    phit = nc.dram_tensor("phit", [rows, C], bf16, kind="ExternalInput")
    cbank = nc.dram_tensor("cbank", [C, M], bf16, kind="ExternalInput")
    ccf = nc.dram_tensor("ccf", [P, M], f32, kind="ExternalInput")
    featv = nc.dram_tensor("featv", [P, nt], f32, kind="ExternalInput")
    out = nc.dram_tensor("out", [rows, 1], f32, kind="ExternalOutput")

    phit_r = phit[:, :].rearrange("(t p) f -> t p f", p=P)
    out_r = out[:, :].rearrange("(p t) o -> p (t o)", t=nt)

    with TileContext(nc) as tc:
        with (
            tc.tile_pool(name="const", bufs=1) as const_pool,
            tc.tile_pool(name="cb", bufs=1) as cb_pool,
            tc.tile_pool(name="lhsT", bufs=lt_bufs) as lhsT_pool,
            tc.tile_pool(name="cand", bufs=cand_bufs) as cand_pool,
            tc.tile_pool(name="mm",
                         bufs=(6 if groups == "kinner" else (mm_rot or 1)),
                         space="PSUM") as mm_pool,
            tc.tile_pool(name="acc", bufs=1) as acc_pool,
            tc.tile_pool(name="fin", bufs=1) as fin_pool,
        ):
            ccf_sb = const_pool.tile([P, M], f32)
            nc.sync.dma_start(ccf_sb[:], ccf[:, :])
            cbt = []
            for k in range(KC):
                ct = cb_pool.tile([P, M], bf16, tag=f"cb{k}")
                nc.sync.dma_start(ct[:, 0:MT], cbank[k * P:(k + 1) * P, 0:MT])
                cbt.append(ct)
            for k in range(KC):
                nc.sync.dma_start(cbt[k][:, MT:], cbank[k * P:(k + 1) * P, MT:])

            feat = acc_pool.tile([P, nt], f32)
            nc.sync.dma_start(feat[:], featv[:, :])
            allv = acc_pool.tile([P, nt * 8], f32)

            def body():
                for t in range(nt):
                    lt = lhsT_pool.tile([P, KC * P], bf16)
                    nc.sync.dma_start(lt[:], phit_r[t])
                    cand = cand_pool.tile([P, NMT * 8], f32)
                    if groups == "kinner":
                        for j in range(NMT):
                            ps = mm_pool.tile([P, MT], f32)
                            for k in range(KC):
                                nc.tensor.matmul(
                                    ps[:],
                                    lhsT=lt[:, k * P:(k + 1) * P],
                                    rhs=cbt[k][:, j * MT:(j + 1) * MT],
                                    start=(k == 0), stop=(k == KC - 1),
                                )
                            nc.vector.tensor_add(
                                ps[:], ps[:], ccf_sb[:, j * MT:(j + 1) * MT])
                            nc.vector.max(out=cand[:, j * 8:(j + 1) * 8],
                                          in_=ps[:])
                    else:
                        pss = [mm_pool.tile([P, MT], f32,
                                            tag=("psr" if mm_rot
                                                 else f"ps{j}"),
                                            name=f"ps{j}_{t}")
                               for j in range(NMT)]
                        j0 = 0
                        for gsz in groups:
                            js = list(range(j0, j0 + gsz))
                            j0 += gsz
                            if centers == "preload":
                                # seed psum with the centers row; matmuls
                                # accumulate on top (start=False), so the
                                # drain is just the max8
                                for j in js:
                                    nc.vector.tensor_copy(
                                        pss[j][:],
                                        ccf_sb[:, j * MT:(j + 1) * MT])
                            for k in range(KC):
                                for j in js:
                                    nc.tensor.matmul(
                                        pss[j][:],
                                        lhsT=lt[:, k * P:(k + 1) * P],
                                        rhs=cbt[k][:, j * MT:(j + 1) * MT],
                                        start=(k == 0 and centers != "preload"),
                                        stop=(k == KC - 1),
                                        skip_group_check=(centers == "preload"),
                                    )
                            for j in js:
                                if centers != "preload":
                                    nc.vector.tensor_add(
                                        pss[j][:], pss[j][:],
                                        ccf_sb[:, j * MT:(j + 1) * MT])
                                nc.vector.max(out=cand[:, j * 8:(j + 1) * 8],
                                              in_=pss[j][:])
                    nc.vector.max(out=allv[:, t * 8:(t + 1) * 8], in_=cand[:])

                # ---- final softmin math
                allv_r = allv[:].rearrange("p (t e) -> p e t", e=8)
                d2 = fin_pool.tile([P, 3 * nt], f32)
                for i in range(3):
                    tmp = fin_pool.tile([P, nt], f32, tag=f"tmp{i}")
                    nc.vector.tensor_scalar_mul(tmp[:], allv_r[:, i, :], 2.0)
                    nc.vector.tensor_sub(d2[:, i * nt:(i + 1) * nt], feat[:], tmp[:])
                d = fin_pool.tile([P, 3 * nt], f32)
                nc.scalar.sqrt(d[:], d2[:])
                g = fin_pool.tile([P, 2 * nt], f32)
                nc.vector.tensor_sub(g[:, :nt], d[:, :nt], d[:, nt:2 * nt])
                nc.vector.tensor_sub(g[:, nt:], d[:, :nt], d[:, 2 * nt:])
                e = fin_pool.tile([P, 2 * nt], f32)
                nc.scalar.activation(e[:], g[:], mybir.ActivationFunctionType.Exp)
                s = fin_pool.tile([P, nt], f32)
                nc.vector.tensor_add(s[:], e[:, :nt], e[:, nt:])
                nc.vector.tensor_scalar_add(s[:], s[:], 1.0)
                r = fin_pool.tile([P, nt], f32)
                nc.vector.reciprocal(r[:], s[:])
                sc = fin_pool.tile([P, nt], f32)
                nc.vector.tensor_mul(sc[:], d[:, :nt], r[:])
                nc.sync.dma_start(out_r, sc[:])

            if reps > 1:
                with tc.For_i(0, reps, 1):
                    body()
            else:
                body()

    return nc


def _build_program5(nt=NT, reps=1, groups=(3, 4), lt_bufs=3, cand_bufs=2,
                    fch=7, mm_bufs=1, bulk_ring="gpsimd"):
    """v5: v4-groups with (a) j-major contiguous C_bank SBUF layout loaded in
    7 one-shot DMAs on the ACT HWDGE ring (SP ring keeps the per-tile lhsT
    loads -> no head-of-line blocking, matmuls start after ~2MB instead of
    11.2MB), and (b) the final softmin math + output DMA chunked every `fch`
    row-tiles so the tail shrinks from ~11us to ~1.5us."""
    import concourse.mybir as mybir
    from concourse import bacc
    from concourse.tile import TileContext

    f32 = mybir.dt.float32
    bf16 = mybir.dt.bfloat16
    rows = P * nt

    nc = bacc.Bacc("TRN2", target_bir_lowering=False, debug=False)
    phit = nc.dram_tensor("phit", [rows, C], bf16, kind="ExternalInput")
    cbj = nc.dram_tensor("cbj", [P, NMT * KC * MT], bf16, kind="ExternalInput")
    ccf = nc.dram_tensor("ccf", [P, M], f32, kind="ExternalInput")
    featv = nc.dram_tensor("featv", [P, nt], f32, kind="ExternalInput")
    out = nc.dram_tensor("out", [rows, 1], f32, kind="ExternalOutput")

    phit_r = phit[:, :].rearrange("(t p) f -> t p f", p=P)
    out_r = out[:, :].rearrange("(p t) o -> p (t o)", t=nt)
    BLK = KC * MT  # one j-block: all 14 k-slices for one 448-wide m-tile

    with TileContext(nc) as tc:
        with (
            tc.tile_pool(name="const", bufs=1) as const_pool,
            tc.tile_pool(name="lhsT", bufs=lt_bufs) as lhsT_pool,
            tc.tile_pool(name="cand", bufs=cand_bufs) as cand_pool,
            tc.tile_pool(name="mm", bufs=mm_bufs, space="PSUM") as mm_pool,
            tc.tile_pool(name="acc", bufs=1) as acc_pool,
            tc.tile_pool(name="fin", bufs=2) as fin_pool,
        ):
            # SP ring: lt[0] first so the first matmul group is unblocked
            # early; then the small f32 constants.
            lt0 = lhsT_pool.tile([P, KC * P], bf16, tag="lt", name="lt_0")
            nc.sync.dma_start(lt0[:], phit_r[0])
            feat = acc_pool.tile([P, nt], f32)
            nc.sync.dma_start(feat[:], featv[:, :])
            ccf_sb = const_pool.tile([P, M], f32)
            nc.sync.dma_start(ccf_sb[:], ccf[:, :])

            # second ring: the 11.2MB bank, one contiguous DMA per j-block.
            bulk = {"gpsimd": nc.gpsimd, "scalar": nc.scalar,
                    "sync": nc.sync}[bulk_ring]
            cb_sb = const_pool.tile([P, NMT * BLK], bf16)
            for j in range(NMT):
                bulk.dma_start(cb_sb[:, j * BLK:(j + 1) * BLK],
                               cbj[:, j * BLK:(j + 1) * BLK])

            allv = acc_pool.tile([P, nt * 8], f32)

            def fin_chunk(t0, t1):
                w = t1 - t0
                allv_r = allv[:].rearrange("p (t e) -> p e t", e=8)
                d2 = fin_pool.tile([P, 3 * w], f32, name=f"d2_{t0}")
                for i in range(3):
                    tmp = fin_pool.tile([P, w], f32, tag=f"tmp{i}",
                                        name=f"tmp{i}_{t0}")
                    nc.vector.tensor_scalar_mul(tmp[:], allv_r[:, i, t0:t1], 2.0)
                    nc.vector.tensor_sub(d2[:, i * w:(i + 1) * w],
                                         feat[:, t0:t1], tmp[:])
                d = fin_pool.tile([P, 3 * w], f32, name=f"d_{t0}")
                nc.scalar.sqrt(d[:], d2[:])
                g = fin_pool.tile([P, 2 * w], f32, name=f"g_{t0}")
                nc.vector.tensor_sub(g[:, :w], d[:, :w], d[:, w:2 * w])
                nc.vector.tensor_sub(g[:, w:], d[:, :w], d[:, 2 * w:])
                e = fin_pool.tile([P, 2 * w], f32, name=f"e_{t0}")
                nc.scalar.activation(e[:], g[:], mybir.ActivationFunctionType.Exp)
                s = fin_pool.tile([P, w], f32, name=f"s_{t0}")
                nc.vector.tensor_add(s[:], e[:, :w], e[:, w:])
                nc.vector.tensor_scalar_add(s[:], s[:], 1.0)
                r = fin_pool.tile([P, w], f32, name=f"r_{t0}")
                nc.vector.reciprocal(r[:], s[:])
                sc = fin_pool.tile([P, w], f32, name=f"sc_{t0}")
                nc.vector.tensor_mul(sc[:], d[:, :w], r[:])
                nc.sync.dma_start(out_r[:, t0:t1], sc[:])

            def body():
                for t in range(nt):
                    if t == 0:
                        lt = lt0
                    else:
                        lt = lhsT_pool.tile([P, KC * P], bf16, tag="lt",
                                            name=f"lt_{t}")
                        nc.sync.dma_start(lt[:], phit_r[t])
                    cand = cand_pool.tile([P, NMT * 8], f32)
                    if groups == "kinner":
                        for j in range(NMT):
                            ps = mm_pool.tile([P, MT], f32)
                            for k in range(KC):
                                nc.tensor.matmul(
                                    ps[:],
                                    lhsT=lt[:, k * P:(k + 1) * P],
                                    rhs=cb_sb[:, (j * KC + k) * MT:
                                              (j * KC + k + 1) * MT],
                                    start=(k == 0), stop=(k == KC - 1),
                                )
                            nc.vector.tensor_add(
                                ps[:], ps[:], ccf_sb[:, j * MT:(j + 1) * MT])
                            nc.vector.max(out=cand[:, j * 8:(j + 1) * 8],
                                          in_=ps[:])
                    else:
                        pss = [mm_pool.tile([P, MT], f32, tag=f"ps{j}",
                                            name=f"ps{j}_{t}")
                               for j in range(NMT)]
                        j0 = 0
                        for gsz in groups:
                            js = list(range(j0, j0 + gsz))
                            j0 += gsz
                            for k in range(KC):
                                for j in js:
                                    nc.tensor.matmul(
                                        pss[j][:],
                                        lhsT=lt[:, k * P:(k + 1) * P],
                                        rhs=cb_sb[:, (j * KC + k) * MT:
                                                  (j * KC + k + 1) * MT],
                                        start=(k == 0), stop=(k == KC - 1),
                                    )
                            for j in js:
                                nc.vector.tensor_add(
                                    pss[j][:], pss[j][:],
                                    ccf_sb[:, j * MT:(j + 1) * MT])
                                nc.vector.max(out=cand[:, j * 8:(j + 1) * 8],
                                              in_=pss[j][:])
                    nc.vector.max(out=allv[:, t * 8:(t + 1) * 8], in_=cand[:])
                    if fch and (t + 1) % fch == 0:
                        fin_chunk(t + 1 - fch, t + 1)
                if not fch:
                    fin_chunk(0, nt)
                elif nt % fch:
                    fin_chunk(nt - nt % fch, nt)

            if reps > 1:
                with tc.For_i(0, reps, 1):
                    body()
            else:
                body()

    return nc


def _host_prep_cbj(C_bank):
    """C_bank [C, M] f32 -> bf16 j-major SBUF layout [P, NMT*KC*MT] with
    cbj[p, (j*KC + k)*MT + mm] = C_bank[k*128 + p, j*448 + mm]."""
    import ml_dtypes
    x = C_bank.astype(ml_dtypes.bfloat16)
    x = x.reshape(KC, P, NMT, MT).transpose(1, 2, 0, 3)   # [p, j, k, mm]
    return np.ascontiguousarray(x.reshape(P, NMT * KC * MT))


def _make_in_maps5(phi_p, C_bank):
    cbj = _host_prep_cbj(C_bank)
    row = -0.5 * (C_bank.astype(np.float64) ** 2).sum(0)
    ccf = np.ascontiguousarray(
        np.broadcast_to(row.astype(np.float32), (P, M)))
    phi2 = np.ascontiguousarray(phi_p.reshape(B * HW, C))
    in_maps = []
    for k in range(NCORES):
        pc = phi2[k * ROWS:(k + 1) * ROWS]
        in_maps.append({"phit": _host_prep_phit(pc), "cbj": cbj,
                        "ccf": ccf, "featv": _host_feat(pc)})
    return in_maps


def _make_in_maps4(phi_p, C_bank):
    import ml_dtypes
    cb_bf = np.ascontiguousarray(C_bank.astype(ml_dtypes.bfloat16))
    row = -0.5 * (C_bank.astype(np.float64) ** 2).sum(0)
    ccf = np.ascontiguousarray(
        np.broadcast_to(row.astype(np.float32), (P, M)))
    phi2 = np.ascontiguousarray(phi_p.reshape(B * HW, C))
    in_maps = []
    for k in range(NCORES):
        pc = phi2[k * ROWS:(k + 1) * ROWS]
        in_maps.append({"phit": _host_prep_phit(pc), "cbank": cb_bf,
                        "ccf": ccf, "featv": _host_feat(pc)})
    return in_maps


def _host_prep_phit(phi_core, nt=NT):
    """[rows, C] f32 -> [nt*P, KC*P] bf16, laid out so lhsT tile t is one
    contiguous 448KB block: phit[t*128 + p', k*128 + n'] = phi[t*128 + n', k*128 + p']."""
    import ml_dtypes
    # tile t, sbuf partition p' (= contraction c_local), free n' (= within-tile
    # row index); within-tile row n' maps to phi row n'*nt + t (v1 mapping).
    x = phi_core.reshape(P, nt, KC, P).transpose(1, 3, 2, 0)   # [t, p', k, n']
    return np.ascontiguousarray(x.reshape(nt * P, KC * P).astype(ml_dtypes.bfloat16))


def _host_prep(C_bank):
    import ml_dtypes
    bf = ml_dtypes.bfloat16
    cb_bf = np.ascontiguousarray(C_bank.astype(bf))
    row = -0.5 * (C_bank.astype(np.float64) ** 2).sum(0)
    chi = row.astype(np.float32).astype(bf)
    clo = (row - chi.astype(np.float64)).astype(np.float32).astype(bf)
    cc2 = np.ascontiguousarray(np.stack([chi, clo]))
    ccf = np.ascontiguousarray(
        np.broadcast_to(row.astype(np.float32), (P, C_bank.shape[1])))
    return cb_bf, cc2, ccf


def _host_prep_cb8(C_bank):
    """C_bank [C, M] f32 -> (cba, cbb) fp8 e4m3 hi/lo in k-major SBUF layout
    [P, KC*M] with cb[p, k*M + m] = x[k*128 + p, m], plus ccf [P, M] f32
    broadcast of the -0.5|c_m|^2 row."""
    import ml_dtypes
    e4 = ml_dtypes.float8_e4m3
    hi = C_bank.astype(e4)
    lo = (C_bank - hi.astype(np.float32)).astype(e4)

    def lay(x):
        return np.ascontiguousarray(
            x.reshape(KC, P, M).transpose(1, 0, 2).reshape(P, KC * M))

    row = -0.5 * (C_bank.astype(np.float64) ** 2).sum(0)
    ccf = np.ascontiguousarray(
        np.broadcast_to(row.astype(np.float32), (P, M)))
    return lay(hi), lay(lo), ccf


def _host_prep_phit8(phi_core, nt=NT):
    """[rows, C] f32 -> (hi, lo) fp8 e4m3 in the transposed lhsT layout
    phit[t*128 + p', k*128 + n'] = phi[p... row n'*nt + t, k*128 + p']."""
    import ml_dtypes
    e4 = ml_dtypes.float8_e4m3
    x = phi_core.reshape(P, nt, KC, P).transpose(1, 3, 2, 0)
    x = np.ascontiguousarray(x.reshape(nt * P, KC * P))
    hi = x.astype(e4)
    lo = (x - hi.astype(np.float32)).astype(e4)
    return hi, lo


def _host_feat(phi_core, nt=NT):
    """[rows, C] f32 -> [P, nt] f32 of |phi_row|^2, row = p*nt + t."""
    return np.einsum("rc,rc->r", phi_core, phi_core).reshape(P, nt)


def _make_in_maps3(phi_p, C_bank):
    cba, cbb, ccf = _host_prep_cb8(C_bank)
    phi2 = np.ascontiguousarray(phi_p.reshape(B * HW, C))
    in_maps = []
    for k in range(NCORES):
        pc = phi2[k * ROWS:(k + 1) * ROWS]
        pa, pb = _host_prep_phit8(pc)
        in_maps.append({"phita": pa, "phitb": pb, "cba": cba, "cbb": cbb,
                        "ccf": ccf, "featv": _host_feat(pc)})
    return in_maps


def kernel(phi_p: np.ndarray, C_bank: np.ndarray) -> np.ndarray:
    from concourse.bass_utils import run_bass_kernel_spmd

    if "nc" not in _CACHE:
        nc = _build_program4(**KERNEL_KW)
        nc.finalize()
        _CACHE["nc"] = nc
    nc = _CACHE["nc"]

    phi_p = np.asarray(phi_p, dtype=np.float32)
    C_bank = np.asarray(C_bank, dtype=np.float32)
    in_maps = _make_in_maps4(phi_p, C_bank)
    res = None
    for attempt in range(3):
        try:
            res = run_bass_kernel_spmd(nc, in_maps, list(range(NCORES)))
            break
        except Exception:
            # transient NRT device errors have been observed; reset the jax
            # backend connection and retry
            if attempt == 2:
                raise
            import time as _time
            _time.sleep(5)
            try:
                import jax
                jax.clear_caches()
                jax.extend.backend.clear_backends()
            except Exception:
                pass
    out = np.concatenate([res.results[k]["out"] for k in range(NCORES)], axis=0)
    return out.reshape(B, HW, 1)

